# revision 8
# baseline (speedup 1.0000x reference)
"""BinaryBasicBlock Trainium2 kernel (8-core SPMD, data-parallel over batch).

Reference computation (per problem statement):
  out1 = relu(BN1(binconv(x, w1) * alpha1 * beta1))
  out  = relu(BN2(binconv(out1, w2) * alpha2 * beta2) + x)

where binconv centers the input per (n,c) over spatial dims, takes sign, and
convolves with sign(w) (3x3, stride 1, pad 1); beta = mean |centered input|
over the WHOLE batch (cross-core all-reduce); BN uses batch statistics over
(N, H, W) (cross-core all-reduce).

Implementation notes:
  - signs are +-1 (exact 0 on knife-edge), so the conv is computed in fp8
    (e4m3, exact for +-1/0) with DoubleRow perf mode: K=256 per matmul,
    fp32 PSUM accumulation => conv outputs are exact integers.
  - conv is 9 shifted matmuls over a zero-padded [58x58] "slab" layout; each
    PSUM tile covers 8 output rows x 58 cols (464 <= 512, one PSUM bank);
    2 junk columns per row are never read back.
  - counts are stored as fp16 half-counts (|count|<=2304, counts even,
    so count/2 <= 1152 is fp16-exact).
  - BN statistics via bn_stats/bn_aggr on the half-counts; BN applied as
    out = A[c]*halfcnt + B[c] with A,B computed on-chip from the two
    all-reduces (beta sums merged into the BN-stats all-reduce).
  - work is spread over ACT/DVE/GpSimd so the PE matmul stream stays the
    critical path; activation tables preloaded at head.
"""

import sys

sys.path.insert(0, "/opt/trn_rl_repo")

import numpy as np

import concourse.bass as bass
import concourse.bacc as bacc
import concourse.tile as tile
import concourse.mybir as mybir
from concourse import bass_isa
from concourse import bass_utils

# ---------------------------------------------------------------- constants
N_CORES = 8
NIMG = 4          # images per core (32 / 8)
C = 256
P = 128
CT = 2            # channel tiles (256 / 128)
H = W = 56
HW = H * W        # 3136
PADW = 58
SLAB = 3392       # padded-slab stride (>= 58*58+2, 16-aligned)
RG_ROWS = 8       # output rows per PSUM tile
NRG = 7           # row groups per image (56 / 8)
NFREE = RG_ROWS * PADW   # 464 (<= 512, one PSUM bank)
NVAL = RG_ROWS * W       # 448 valid outputs per PSUM tile
NTAP = 9
EPS = 1e-5
NTOT = 32 * C * HW       # global element count for beta = mean|xc|
NCH = 32 * HW            # global per-channel count for BN stats
NLOC = NIMG * HW         # per-core per-channel count

F32 = mybir.dt.float32
F16 = mybir.dt.float16
BF16 = mybir.dt.bfloat16
FP8 = mybir.dt.float8e4
U32 = mybir.dt.uint32

FP8_NP = mybir.dt.np(FP8)

AX = mybir.AxisListType
ALU = mybir.AluOpType
ACTF = mybir.ActivationFunctionType


def _rhs_off(rg: int, dy: int, dx: int) -> int:
    # output rows y0..y0+7; rhs element j maps to padded input
    # [(y0+1+dy)*58 + 1 + dx] + j
    return (rg * RG_ROWS + 1 + dy) * PADW + 1 + dx


def _conv_img(nc, psum, wall, slab, cnt_n, bnst, n, cv_tag):
    """One image of one binary conv: 9-tap DoubleRow matmuls + evacuation
    + bn_stats partials over the fp16 half-counts."""
    w5 = wall.rearrange("p (m t j c) -> p m t j c", m=CT, t=NTAP, j=CT)
    if True:
        if True:
            slab3 = slab.rearrange("p (j s) -> p j s", j=CT)
            for m in range(CT):
                ptiles = []
                for rg in range(NRG):
                    ptile = psum.tile([P, NFREE], F32,
                                      name=f"pt_{cv_tag}_{n}_{m}_{rg}", tag="pt")
                    ptiles.append(ptile)
                for tap in range(NTAP):
                    dy, dx = tap // 3 - 1, tap % 3 - 1
                    for rg in range(NRG):
                        off = _rhs_off(rg, dy, dx)
                        nc.tensor.matmul(
                            ptiles[rg][:, :],
                            lhsT=w5[:, m, tap],
                            rhs=slab3[:, :, off:off + NFREE],
                            start=(tap == 0),
                            stop=(tap == NTAP - 1),
                            perf_mode=mybir.MatmulPerfMode.DoubleRow,
                        )
                for rg in range(NRG):
                    pv = ptiles[rg].rearrange("p (r x) -> p r x", x=PADW)[:, :, 0:W]
                    cslice = cnt_n[:, m * HW + rg * NVAL: m * HW + (rg + 1) * NVAL]
                    cv = cslice.rearrange("p (r x) -> p r x", x=W)
                    col = n * NRG + rg
                    # evacuate as half-counts (exact in fp16)
                    nc.scalar.activation(cv, pv, ACTF.Copy, bias=0.0, scale=0.5)
                    # per-channel partial stats of the half-counts
                    nc.vector.bn_stats(
                        bnst[:, (m * 28 + col) * 6: (m * 28 + col + 1) * 6],
                        cslice,
                    )


def _center_sign(nc, src_view, slab2, t, negm):
    """sign(src - mean) into padded slab tile t; returns the sign view."""
    interior = slab2[:, t * SLAB + PADW + 1: t * SLAB + PADW + 1 + 56 * PADW]
    sview = interior.rearrange("p (r x) -> p r x", x=PADW)[:, :, 0:W]
    nc.scalar.activation(sview, src_view, ACTF.Sign, bias=negm[:, :])
    return sview


def _bn_coeffs(nc, arres, alpha_sb, gamma_sb, bnb_sb, cpool, tag):
    """From all-reduced [beta_sum(partition-summed), sum0, sum1, sumsq0,
    sumsq1] compute A = 2*s*gamma*rsqrt(4*s^2*v + eps), B = bn_beta - A*mean
    per channel. Returns (A, B) tiles of shape [P, CT]."""
    s = cpool.tile([P, 1], F32, name=f"s_{tag}", tag=f"s_{tag}")
    # s = alpha * beta = alpha * beta_sum / NTOT
    nc.vector.tensor_scalar(s[:, :], arres[:, 0:1], alpha_sb[:, 0:1], 1.0 / NTOT,
                            op0=ALU.mult, op1=ALU.mult)
    s2 = cpool.tile([P, 1], F32, name=f"s2_{tag}", tag=f"s2_{tag}")
    nc.vector.tensor_scalar_mul(s2[:, :], s[:, :], 2.0)
    q4 = cpool.tile([P, 1], F32, name=f"q4_{tag}", tag=f"q4_{tag}")
    nc.vector.tensor_scalar(q4[:, :], s[:, :], s[:, 0:1], 4.0,
                            op0=ALU.mult, op1=ALU.mult)
    m_h = cpool.tile([P, CT], F32, name=f"mh_{tag}", tag=f"mh_{tag}")
    nc.vector.tensor_scalar(m_h[:, :], arres[:, 1:3], 1.0 / NCH, None, op0=ALU.mult)
    ex2 = cpool.tile([P, CT], F32, name=f"ex2_{tag}", tag=f"ex2_{tag}")
    nc.vector.tensor_scalar(ex2[:, :], arres[:, 3:5], 1.0 / NCH, None, op0=ALU.mult)
    msq = cpool.tile([P, CT], F32, name=f"msq_{tag}", tag=f"msq_{tag}")
    nc.vector.tensor_tensor(msq[:, :], m_h[:, :], m_h[:, :], op=ALU.mult)
    v_h = cpool.tile([P, CT], F32, name=f"vh_{tag}", tag=f"vh_{tag}")
    nc.vector.tensor_tensor(v_h[:, :], ex2[:, :], msq[:, :], op=ALU.subtract)
    arg = cpool.tile([P, CT], F32, name=f"arg_{tag}", tag=f"arg_{tag}")
    nc.vector.tensor_scalar(arg[:, :], v_h[:, :], q4[:, 0:1], EPS,
                            op0=ALU.mult, op1=ALU.add)
    lnv = cpool.tile([P, CT], F32, name=f"lnv_{tag}", tag=f"lnv_{tag}")
    nc.scalar.activation(lnv[:, :], arg[:, :], ACTF.Ln)
    rsq = cpool.tile([P, CT], F32, name=f"rsq_{tag}", tag=f"rsq_{tag}")
    nc.scalar.activation(rsq[:, :], lnv[:, :], ACTF.Exp, scale=-0.5)
    A = cpool.tile([P, CT], F32, name=f"A_{tag}", tag=f"A_{tag}")
    # A = (rsq * 2s) * gamma
    nc.vector.scalar_tensor_tensor(A[:, :], in0=rsq[:, :], scalar=s2[:, 0:1],
                                   in1=gamma_sb[:, :], op0=ALU.mult,
                                   op1=ALU.mult)
    amh = cpool.tile([P, CT], F32, name=f"amh_{tag}", tag=f"amh_{tag}")
    nc.vector.tensor_tensor(amh[:, :], A[:, :], m_h[:, :], op=ALU.mult)
    B = cpool.tile([P, CT], F32, name=f"B_{tag}", tag=f"B_{tag}")
    nc.vector.tensor_tensor(B[:, :], bnb_sb[:, :], amh[:, :], op=ALU.subtract)
    return A, B


def build_nc():
    nc = bacc.Bacc("TRN2", target_bir_lowering=False, debug=False,
                   num_devices=N_CORES)

    x_d = nc.dram_tensor("x", [NIMG, C, H, W], F32, kind="ExternalInput")
    WSZ = CT * NTAP * CT * P  # 4608
    w1_d = nc.dram_tensor("w1", [P, WSZ], FP8, kind="ExternalInput")
    w2_d = nc.dram_tensor("w2", [P, WSZ], FP8, kind="ExternalInput")
    g1_d = nc.dram_tensor("g1", [P, CT], F32, kind="ExternalInput")
    b1_d = nc.dram_tensor("b1", [P, CT], F32, kind="ExternalInput")
    g2_d = nc.dram_tensor("g2", [P, CT], F32, kind="ExternalInput")
    b2_d = nc.dram_tensor("b2", [P, CT], F32, kind="ExternalInput")
    a1_d = nc.dram_tensor("a1", [P, 1], F32, kind="ExternalInput")
    a2_d = nc.dram_tensor("a2", [P, 1], F32, kind="ExternalInput")
    out_d = nc.dram_tensor("out", [NIMG, C, H, W], F32, kind="ExternalOutput")

    with tile.TileContext(nc) as tc:
        with tc.tile_pool(name="persist", bufs=1) as persist, \
             tc.tile_pool(name="xio", bufs=3) as xio, \
             tc.tile_pool(name="r1p", bufs=3) as r1p, \
             tc.tile_pool(name="scrp", bufs=2) as scrp, \
             tc.tile_pool(name="outp", bufs=3) as outp, \
             tc.tile_pool(name="small", bufs=12) as small, \
             tc.tile_pool(name="psum", bufs=8, space="PSUM") as psum, \
             tc.tile_pool(name="dram", bufs=1, space="DRAM") as dram:

            # ---- activation-table preload (natural_log_exp set has ln/exp
            # as anchors and copy/sign/abs/relu as fillers): one dummy Ln+Exp
            # at the head keeps ACT_TABLE_LOADs off the critical path.
            dumm = persist.tile([P, 1], F32, tag="dumm")
            nc.vector.memset(dumm[:, :], 1.0)
            nc.scalar.activation(dumm[:, :], dumm[:, :], ACTF.Ln)
            nc.scalar.activation(dumm[:, :], dumm[:, :], ACTF.Exp, scale=0.0)

            # ---- first image loads get the sync DMA ring first
            xa00 = xio.tile([P, HW], F32, name="xa00", tag="xio")
            nc.sync.dma_start(out=xa00[:, :], in_=x_d.ap()[0, 0:P])
            xa01 = xio.tile([P, HW], F32, name="xa01", tag="xio")
            nc.sync.dma_start(out=xa01[:, :], in_=x_d.ap()[0, P:2 * P])

            # ---- weights + params on the scalar (ACT) HWDGE ring
            w1sb = persist.tile([P, WSZ], FP8, tag="w1sb")
            w2sb = persist.tile([P, WSZ], FP8, tag="w2sb")
            nc.scalar.dma_start(out=w1sb[:, :], in_=w1_d.ap())
            nc.scalar.dma_start(out=w2sb[:, :], in_=w2_d.ap())
            g1sb = persist.tile([P, CT], F32, tag="g1sb")
            b1sb = persist.tile([P, CT], F32, tag="b1sb")
            g2sb = persist.tile([P, CT], F32, tag="g2sb")
            b2sb = persist.tile([P, CT], F32, tag="b2sb")
            a1sb = persist.tile([P, 1], F32, tag="a1sb")
            a2sb = persist.tile([P, 1], F32, tag="a2sb")
            for sb, d in ((g1sb, g1_d), (b1sb, b1_d), (g2sb, g2_d),
                          (b2sb, b2_d), (a1sb, a1_d), (a2sb, a2_d)):
                nc.scalar.dma_start(out=sb[:, :], in_=d.ap())

            # ---- persistent per-image buffers (pad zeros via gpsimd memset)
            slabs = [persist.tile([P, CT * SLAB], FP8, name=f"slab_{n}",
                                  tag=f"slab_{n}") for n in range(NIMG)]
            cnt = [persist.tile([P, CT * HW], F16, name=f"cnt_{n}",
                                tag=f"cnt_{n}") for n in range(NIMG)]
            for n in range(NIMG):
                nc.gpsimd.memset(slabs[n][:, :].bitcast(U32), 0)

            # ---- stats buffers
            beta1_parts = persist.tile([P, NIMG * CT], F32, tag="beta1_parts")
            beta2_parts = persist.tile([P, NIMG * CT], F32, tag="beta2_parts")
            bnst1 = persist.tile([P, CT * 28 * 6], F32, tag="bnst1")
            bnst2 = persist.tile([P, CT * 28 * 6], F32, tag="bnst2")
            aggr1 = persist.tile([P, CT, 2], F32, tag="aggr1")
            aggr2 = persist.tile([P, CT, 2], F32, tag="aggr2")
            mm1 = persist.tile([P, CT], F32, tag="mm1")
            mm2 = persist.tile([P, CT], F32, tag="mm2")
            ex1 = persist.tile([P, CT], F32, tag="ex1")
            ex2b = persist.tile([P, CT], F32, tag="ex2b")
            arbuf1 = persist.tile([P, 5], F32, tag="arbuf1")
            arres1 = persist.tile([P, 5], F32, tag="arres1")
            arbuf2 = persist.tile([P, 5], F32, tag="arbuf2")
            arres2 = persist.tile([P, 5], F32, tag="arres2")
            bred1 = persist.tile([P, 1], F32, tag="bred1")
            bred2 = persist.tile([P, 1], F32, tag="bred2")
            ar1_in = dram.tile([P, 5], F32, tag="ar1_in")
            ar1_out = dram.tile([P, 5], F32, tag="ar1_out")
            ar2_in = dram.tile([P, 5], F32, tag="ar2_in")
            ar2_out = dram.tile([P, 5], F32, tag="ar2_out")

            # ======= stage A + conv1, interleaved per image so the ACT
            # queue never has later-image prep ahead of PSUM evacuations
            for n in range(NIMG):
                absq = []
                for t in range(CT):
                    if n == 0:
                        xa = xa00 if t == 0 else xa01
                    else:
                        xa = xio.tile([P, HW], F32, name=f"xa_{n}_{t}", tag="xio")
                        nc.sync.dma_start(out=xa[:, :],
                                          in_=x_d.ap()[n, t * P:(t + 1) * P])
                    sums = small.tile([P, 1], F32, name=f"sA_{n}_{t}", tag="sm")
                    nc.vector.tensor_reduce(sums[:, :], xa[:, :], axis=AX.X,
                                            op=ALU.add)
                    negm = small.tile([P, 1], F32, name=f"nA_{n}_{t}", tag="nm")
                    nc.vector.tensor_scalar_mul(negm[:, :], sums[:, :], -1.0 / HW)
                    xv = xa.rearrange("p (r x) -> p r x", x=W)
                    _center_sign(nc, xv, slabs[n], t, negm)
                    absq.append((xv, negm, t))
                for xv, negm, t in absq:
                    scr = scrp.tile([P, H, W], FP8, name=f"scrA_{n}_{t}", tag="scr")
                    nc.scalar.activation(
                        scr[:, :, :], xv, ACTF.Abs, bias=negm[:, :],
                        accum_out=beta1_parts[:, n * CT + t: n * CT + t + 1])
                _conv_img(nc, psum, w1sb, slabs[n], cnt[n], bnst1, n, "c1")
            # partition-sum the beta1 partials (off the AR critical path)
            nc.vector.tensor_reduce(bred1[:, :], beta1_parts[:, :], axis=AX.X,
                                    op=ALU.add)
            nc.gpsimd.partition_all_reduce(arbuf1[:, 0:1], bred1[:, :],
                                           channels=P,
                                           reduce_op=bass_isa.ReduceOp.add)

            # ================= all-reduce 1 (beta1 + BN1 stats)
            for m in range(CT):
                nc.vector.bn_aggr(aggr1[:, m, :], bnst1[:, m * 168:(m + 1) * 168])
            # sum = N*mean ; sumsq = N*(var + mean^2)
            nc.vector.tensor_scalar(arbuf1[:, 1:3], aggr1[:, :, 0], float(NLOC),
                                    None, op0=ALU.mult)
            nc.vector.tensor_tensor(mm1[:, :], aggr1[:, :, 0], aggr1[:, :, 0],
                                    op=ALU.mult)
            nc.vector.tensor_tensor(ex1[:, :], aggr1[:, :, 1], mm1[:, :],
                                    op=ALU.add)
            nc.vector.tensor_scalar(arbuf1[:, 3:5], ex1[:, :], float(NLOC),
                                    None, op0=ALU.mult)
            nc.sync.dma_start(out=ar1_in[:, :], in_=arbuf1[:, :])
            nc.gpsimd.collective_compute(
                "AllReduce", ALU.add, replica_groups=[list(range(N_CORES))],
                ins=[ar1_in.opt()], outs=[ar1_out.opt()])
            nc.sync.dma_start(out=arres1[:, :], in_=ar1_out[:, :])

            A1, B1 = _bn_coeffs(nc, arres1, a1sb, g1sb, b1sb, persist, "bn1")

            # ======= stage C + conv2, interleaved per image
            for n in range(NIMG):
                absq = []
                for t in range(CT):
                    r1 = r1p.tile([P, HW], F32, name=f"r1_{n}_{t}", tag="r1")
                    nc.scalar.activation(r1[:, :], cnt[n][:, t * HW:(t + 1) * HW],
                                         ACTF.Relu, bias=B1[:, t:t + 1],
                                         scale=A1[:, t:t + 1])
                    sums = small.tile([P, 1], F32, name=f"sC_{n}_{t}", tag="sm")
                    nc.vector.tensor_reduce(sums[:, :], r1[:, :], axis=AX.X,
                                            op=ALU.add)
                    negm = small.tile([P, 1], F32, name=f"nC_{n}_{t}", tag="nm")
                    nc.vector.tensor_scalar_mul(negm[:, :], sums[:, :], -1.0 / HW)
                    rv = r1.rearrange("p (r x) -> p r x", x=W)
                    sview = _center_sign(nc, rv, slabs[n], t, negm)
                    absq.append((rv, negm, sview, t))
                for rv, negm, sview, t in absq:
                    # |r1 - m| = (r1 - m) * sign(r1 - m), summed on DVE
                    scr = scrp.tile([P, H, W], FP8, name=f"scrC_{n}_{t}", tag="scr")
                    nc.vector.scalar_tensor_tensor(
                        scr[:, :, :], in0=rv, scalar=negm[:, 0:1], in1=sview,
                        op0=ALU.add, op1=ALU.mult,
                        accum_out=beta2_parts[:, n * CT + t: n * CT + t + 1])
                _conv_img(nc, psum, w2sb, slabs[n], cnt[n], bnst2, n, "c2")
            nc.vector.tensor_reduce(bred2[:, :], beta2_parts[:, :], axis=AX.X,
                                    op=ALU.add)
            nc.gpsimd.partition_all_reduce(arbuf2[:, 0:1], bred2[:, :],
                                           channels=P,
                                           reduce_op=bass_isa.ReduceOp.add)

            # residual prefetch into recycled stage-A/stage-C slots
            # (r1p and xio slots are dead by now; overlaps conv2 tail)
            xres = []
            for n in range(NIMG):
                row = []
                for t in range(CT):
                    k = n * CT + t
                    pool, ptag = (r1p, "r1") if k % 2 == 0 else (xio, "xio")
                    xr = pool.tile([P, HW], F32, name=f"xr_{n}_{t}", tag=ptag)
                    nc.sync.dma_start(out=xr[:, :],
                                      in_=x_d.ap()[n, t * P:(t + 1) * P])
                    row.append(xr)
                xres.append(row)

            # ================= all-reduce 2 (beta2 + BN2 stats)
            for m in range(CT):
                nc.vector.bn_aggr(aggr2[:, m, :], bnst2[:, m * 168:(m + 1) * 168])
            nc.vector.tensor_scalar(arbuf2[:, 1:3], aggr2[:, :, 0], float(NLOC),
                                    None, op0=ALU.mult)
            nc.vector.tensor_tensor(mm2[:, :], aggr2[:, :, 0], aggr2[:, :, 0],
                                    op=ALU.mult)
            nc.vector.tensor_tensor(ex2b[:, :], aggr2[:, :, 1], mm2[:, :],
                                    op=ALU.add)
            nc.vector.tensor_scalar(arbuf2[:, 3:5], ex2b[:, :], float(NLOC),
                                    None, op0=ALU.mult)
            nc.sync.dma_start(out=ar2_in[:, :], in_=arbuf2[:, :])
            nc.gpsimd.collective_compute(
                "AllReduce", ALU.add, replica_groups=[list(range(N_CORES))],
                ins=[ar2_in.opt()], outs=[ar2_out.opt()])
            nc.sync.dma_start(out=arres2[:, :], in_=ar2_out[:, :])

            A2, B2 = _bn_coeffs(nc, arres2, a2sb, g2sb, b2sb, persist, "bn2")

            # ================= final: out = relu(A2*h2 + B2 + x)
            for n in range(NIMG):
                for t in range(CT):
                    z = outp.tile([P, HW], F32, name=f"z_{n}_{t}", tag="z")
                    nc.vector.tensor_scalar(z[:, :], cnt[n][:, t * HW:(t + 1) * HW],
                                            A2[:, t:t + 1], B2[:, t:t + 1],
                                            op0=ALU.mult, op1=ALU.add)
                    nc.vector.tensor_tensor(z[:, :], z[:, :], xres[n][t][:, :],
                                             op=ALU.add)
                    nc.scalar.activation(z[:, :], z[:, :], ACTF.Relu)
                    nc.sync.dma_start(out=out_d.ap()[n, t * P:(t + 1) * P],
                                      in_=z[:, :])

    nc.compile()
    return nc


_NC_CACHE = None


def _get_nc():
    global _NC_CACHE
    if _NC_CACHE is None:
        _NC_CACHE = build_nc()
    return _NC_CACHE


def _pack_w(w: np.ndarray) -> np.ndarray:
    # [Cout, Cin, 3, 3] -> lhsT [128(k), CT(m), 9(tap), CT(j), 128(cout_inner)]
    ws = np.sign(w.astype(np.float32))
    ws = ws.reshape(CT, P, CT, P, NTAP // 3, 3)  # m, cout_in, j, k, ky, kx
    # -> k, m, (ky kx), j, cout_in
    ws = ws.transpose(3, 0, 4, 5, 2, 1).reshape(P, CT * NTAP * CT * P)
    return np.ascontiguousarray(ws).astype(FP8_NP)


def _pack_ch(v: np.ndarray) -> np.ndarray:
    # [256] -> [128, CT] (partition-major within each channel tile)
    return np.ascontiguousarray(np.asarray(v, np.float32).reshape(CT, P).T)


def kernel(x, conv1_w, alpha1, bn1_gamma, bn1_beta, conv2_w, alpha2,
           bn2_gamma, bn2_beta):
    nc = _get_nc()
    x = np.asarray(x, np.float32)
    w1p = _pack_w(np.asarray(conv1_w))
    w2p = _pack_w(np.asarray(conv2_w))
    g1 = _pack_ch(bn1_gamma)
    b1 = _pack_ch(bn1_beta)
    g2 = _pack_ch(bn2_gamma)
    b2 = _pack_ch(bn2_beta)
    a1 = np.full((P, 1), np.float32(np.asarray(alpha1)), np.float32)
    a2 = np.full((P, 1), np.float32(np.asarray(alpha2)), np.float32)

    in_maps = []
    for i in range(N_CORES):
        in_maps.append({
            "x": np.ascontiguousarray(x[i * NIMG:(i + 1) * NIMG]),
            "w1": w1p, "w2": w2p,
            "g1": g1, "b1": b1, "g2": g2, "b2": b2,
            "a1": a1, "a2": a2,
        })
    res = bass_utils.run_bass_kernel_spmd(nc, in_maps,
                                          core_ids=list(range(N_CORES)))
    out = np.concatenate([res.results[i]["out"] for i in range(N_CORES)],
                         axis=0)
    return out.astype(np.float32)
